# revision 13
# baseline (speedup 1.0000x reference)
"""Trainium2 Bass kernel for nn_Blur (upfirdn2d 4x4 blur, pad=(2,1)).

Formulation: out[i,j] = sum_{p,q} Kf[p,q] * x[i+p-2, j+q-2]   (Kf = flip(kernel2d))

For each W-tap q (4 taps), the H-convolution is a banded 64x64 matrix
Aq[i,h] = Kf[h-i+2, q].  x is split host-side into bf16 hi + bf16 lo
(x = hi + lo to ~2^-18 relative).  The blur weights ({1,3,9}/64) have <=4
mantissa bits, so every bf16 product is exact in fp32: the PSUM accumulation
reproduces the fp32 conv to ~1e-6 while streaming at bf16 rate.

One image's hi rows (partitions 0-63) and lo rows (64-127) fill the full
K=128 contraction: lhsT_q = [Aq^T; Aq^T] [128, 64] computes Aq@(hi+lo) in a
single matmul with M=64.  Two such matmuls (8 images each via a stride-68
N=512 access pattern) run CONCURRENTLY on disjoint PE column groups
(tile_position (0,0) / (0,64)), halving effective PE time.  The 4 taps
accumulate into one PSUM bank.

DMA: the host pre-transposes into per-batch [128, 1092] bf16 tiles
(partition-major contiguous, 68-stride zero-padded rows so tap windows read
zeros at W boundaries), so every DMA is a plain 128-descriptor line-rate
transfer.  The fp32 output is written back as [128, 512] tiles and
inverse-transposed on the host.

Sharding: the 16*512 = 8192 independent (n,c) images are split into 8
contiguous slabs of 1024 images, one per NeuronCore (data-parallel).
"""

import ml_dtypes
import numpy as np

import concourse.bacc as bacc
import concourse.bass as bass
import concourse.mybir as mybir
import concourse.tile as tile
from concourse.bass_utils import run_bass_kernel_spmd

N_CORES = 8
IMG = 64                      # H = W
N_IMAGES = 16 * 512           # 8192
PER_CORE = N_IMAGES // N_CORES  # 1024
GROUP = 16                    # images per batch
N_BATCH = PER_CORE // GROUP   # 64
PAD_L, PAD_R = 2, 2           # row padding -> stride 68
S = PAD_L + IMG + PAD_R       # 68
HALF_W = 8 * S                # 544 cols per col-group (8 images)
TILE_W = 2 * HALF_W + 4       # 1092 (4 slack: tap q=3 slice bound)
DT = mybir.dt.float32
IN_DT = mybir.dt.bfloat16
NP_IN = ml_dtypes.bfloat16

LAST_RESULTS = None  # BassKernelResults of the most recent run (for test.py)


def _band64(col_weights: np.ndarray) -> np.ndarray:
    """64x64 band matrix B[i,h] = col_weights[h-i+2] (H-conv, pad=(2,1))."""
    b = np.zeros((64, 64), dtype=np.float64)
    for i in range(64):
        for p in range(4):
            h = i + p - 2
            if 0 <= h < 64:
                b[i, h] = col_weights[p]
    return b


def _build_weights(kernel2d: np.ndarray):
    """Returns ([128, 320] bf16 weights, separable: bool).

    Cols [64q:64q+64] hold [Aq^T; Aq^T] for W-tap q (hi rows; lo rows).
    Cols [256:320] hold the H-only band [Ah^T; Ah^T] with Ah built from
    u = kf[:, 0] when kf == outer(u, [1,3,3,1]) (used by the DVE-offload
    path, whose shift-add chain applies exactly [1,3,3,1] along W)."""
    kf = np.flip(np.asarray(kernel2d, dtype=np.float64), (0, 1))
    wts = np.zeros((128, 320), dtype=NP_IN)
    for q in range(4):
        aqt = _band64(kf[:, q]).T
        wts[:64, q * 64:(q + 1) * 64] = aqt.astype(NP_IN)
        wts[64:, q * 64:(q + 1) * 64] = aqt.astype(NP_IN)
    u = kf[:, 0]
    separable = np.allclose(kf, np.outer(u, [1.0, 3.0, 3.0, 1.0]), rtol=1e-6,
                            atol=1e-12)
    if separable:
        u_b = u.astype(NP_IN)
        separable = bool(np.allclose(u_b.astype(np.float64), u, rtol=1e-7))
    if separable:
        aht = _band64(u_b.astype(np.float64)).T
        wts[:64, 256:320] = aht.astype(NP_IN)
        wts[64:, 256:320] = aht.astype(NP_IN)
    return wts, separable


ZS = 67                        # stride of the DVE-path padded fp32 tile
ZW = 8 * ZS + 4                # 540


def _bass_module(separable: bool) -> bass.Bass:
    nc = bacc.Bacc(
        "TRN2",
        target_bir_lowering=False,
        debug=False,
        num_devices=N_CORES,
    )
    x_d = nc.dram_tensor("x", [N_BATCH, 128, TILE_W], IN_DT, kind="ExternalInput")
    w_d = nc.dram_tensor("wts", [128, 320], IN_DT, kind="ExternalInput")
    o_d = nc.dram_tensor("out", [N_BATCH, 128, 512], DT, kind="ExternalOutput")

    with tile.TileContext(nc) as tc:
        with (
            tc.tile_pool(name="const", bufs=1) as cpool,
            tc.tile_pool(name="inp", bufs=10) as ipool,
            tc.tile_pool(name="outp", bufs=8) as opool,
            tc.tile_pool(name="zp", bufs=3) as zpool,
            tc.tile_pool(name="t1", bufs=3) as t1pool,
            tc.tile_pool(name="t2", bufs=3) as t2pool,
            tc.tile_pool(name="psum", bufs=8, space="PSUM") as ppool,
        ):
            w_tile = cpool.tile([128, 320], IN_DT)
            nc.sync.dma_start(w_tile[:], w_d[:])

            for b in range(N_BATCH):
                in_tile = ipool.tile([128, TILE_W], IN_DT)
                nc.sync.dma_start(in_tile[:], x_d[b])

                ps = ppool.tile([128, 512], DT)
                out_tile = opool.tile([128, 512], DT)

                # Offload the W-conv of every 3rd batch to the Vector engine
                # (PE streams each output once instead of 4x; DVE applies the
                # exact [1,3,3,1] chain in fp32).  Balances PE vs DVE load.
                dve_batch = separable and (b % 3 == 2)

                if dve_batch:
                    for cg in range(2):
                        rhs = in_tile[:, cg * HALF_W + PAD_L:
                                      cg * HALF_W + PAD_L + 8 * S].rearrange(
                            "p (g s) -> p g s", s=S
                        )[:, :, 0:IMG]
                        nc.tensor.matmul(
                            ps[cg * 64:(cg + 1) * 64, :],
                            w_tile[:, 256:320],
                            rhs,
                            start=True,
                            stop=True,
                            tile_position=(0, cg * 64),
                        )
                    zp = zpool.tile([128, ZW], DT)
                    nc.gpsimd.memset(zp[:], 0.0)
                    zdst = zp[:, PAD_L:PAD_L + 8 * ZS].rearrange(
                        "p (g s) -> p g s", s=ZS
                    )[:, :, 0:IMG]
                    nc.vector.tensor_copy(zdst, ps[:].rearrange(
                        "p (g w) -> p g w", w=IMG))
                    t1 = t1pool.tile([128, ZW - 2], DT)
                    nc.vector.tensor_add(t1[:], zp[:, 0:ZW - 2], zp[:, 1:ZW - 1])
                    t2 = t2pool.tile([128, ZW - 3], DT)
                    nc.vector.tensor_add(t2[:], t1[:, 0:ZW - 3], t1[:, 1:ZW - 2])
                    src0 = t2[:, 0:8 * ZS].rearrange(
                        "p (g s) -> p g s", s=ZS)[:, :, 0:IMG]
                    src1 = t2[:, 1:8 * ZS + 1].rearrange(
                        "p (g s) -> p g s", s=ZS)[:, :, 0:IMG]
                    nc.vector.tensor_add(out_tile[:].rearrange(
                        "p (g w) -> p g w", w=IMG), src0, src1)
                else:
                    for q in range(4):
                        for cg in range(2):  # concurrent PE column groups
                            base = cg * HALF_W + q
                            rhs = in_tile[:, base:base + 8 * S].rearrange(
                                "p (g s) -> p g s", s=S
                            )[:, :, 0:IMG]
                            nc.tensor.matmul(
                                ps[cg * 64:(cg + 1) * 64, :],
                                w_tile[:, q * 64:(q + 1) * 64],
                                rhs,
                                start=(q == 0),
                                stop=(q == 3),
                                tile_position=(0, cg * 64),
                            )
                    nc.vector.tensor_copy(out_tile[:], ps[:])

                nc.scalar.dma_start(o_d[b], out_tile[:])
    nc.compile()
    return nc


def _host_pack(x: np.ndarray) -> np.ndarray:
    """FULL x (8192,64,64) f32 -> [N_CORES, N_BATCH, 128, TILE_W] bf16.

    Partition dim = (half, h); free dim = (g: 16 images, s: 68)."""
    hi = x.astype(NP_IN)
    lo = (x - hi.astype(np.float32)).astype(NP_IN)
    packed = np.zeros((2, N_IMAGES, IMG, S), dtype=NP_IN)
    packed[0, :, :, PAD_L:PAD_L + IMG] = hi
    packed[1, :, :, PAD_L:PAD_L + IMG] = lo
    v = packed.reshape(2, N_CORES, N_BATCH, GROUP, IMG, S)
    v = v.transpose(1, 2, 0, 4, 3, 5)  # [core, b, half, h, g, s]
    flat = v.reshape(N_CORES, N_BATCH, 128, GROUP * S)
    out = np.zeros((N_CORES, N_BATCH, 128, TILE_W), dtype=NP_IN)
    out[..., : GROUP * S] = flat
    return out


def _host_unpack(tiles: np.ndarray) -> np.ndarray:
    """[N_CORES, N_BATCH, 128, 512] f32 -> (8192, 64, 64) f32.

    Partition dim = (cg, h); free dim = (g: 8, w); img = b*16 + cg*8 + g."""
    v = tiles.reshape(N_CORES, N_BATCH, 2, IMG, 8, IMG)
    v = v.transpose(0, 1, 2, 4, 3, 5)  # [core, b, cg, g, h, w]
    return v.reshape(N_IMAGES, IMG, IMG)


def kernel(x: np.ndarray, kernel: np.ndarray, _trace: bool = False) -> np.ndarray:
    global LAST_RESULTS
    x = np.ascontiguousarray(np.asarray(x, dtype=np.float32))
    n, c, h, w = x.shape
    assert (n, c, h, w) == (16, 512, 64, 64), x.shape

    shards = _host_pack(x.reshape(N_IMAGES, IMG, IMG))
    wts, separable = _build_weights(kernel)
    in_maps = [{"x": shards[i], "wts": wts} for i in range(N_CORES)]

    nc = _bass_module(separable)
    results = run_bass_kernel_spmd(
        nc, in_maps, core_ids=list(range(N_CORES)), trace=_trace
    )
    LAST_RESULTS = results

    tiles = np.stack([r["out"] for r in results.results])
    out = _host_unpack(tiles)
    return np.ascontiguousarray(out.reshape(n, c, h, w)).astype(np.float32)
